# revision 17
# baseline (speedup 1.0000x reference)
"""Trainium2 kernel for nn_MHAttention_15358803050646.

The reference module computes
    qkv = qkv_w @ x + qkv_b          (1x1 conv over channels)
    q, k, v = split(qkv)
    att = softmax(q @ k^T / sqrt(d_k))
    out = einsum('bnqk,bnqd->bnqd', att, v)      # <-- sums att over k
    out = out_w @ out + out_b

The einsum 'bnqk,bnqd->bnqd' multiplies v elementwise by the softmax
row-sum, which is identically 1.  The whole attention block is therefore
the identity on v, and the network collapses algebraically to

    out = out_w @ (v_w @ x + v_b) + out_b = W_eff @ x + b_eff

with v_w = qkv_w[1024:1536], v_b = qkv_b[1024:1536].  We fuse the two
channel matrices on the host (512x512x512 fp32, sub-millisecond) and run
a single 512x512 channel projection over all pixels on device.  The bias
b_eff is added on the host after the gather: the device output is
rounded to fp16 either way, and |b| << |y|, so adding it before or after
the rounding changes nothing at the 2e-2 tolerance.

Sharding: data-parallel over batch — B == 8 images, one per NeuronCore.
Per core: out[o, p] = sum_c W_eff[o, c] * x[c, p] with C = 512 channels
and HW = 1024 pixels, i.e. a 512x512x1024 matmul.

Precision: the harness tolerance is rel_err < 2e-2; a single fp16
matmul term (fp32 PSUM accumulation) gives ~5e-4, so W and x ship as
fp16 and the PE does one pass (16384 rows).  Outputs return as fp16
(exactly upcast on the host), halving writeback DMA.

Schedule (cost-model driven):
- Phase A = pixels 0..511, k-outer: 4 PSUM groups accumulate across the
  4 k-chunks as they stream in; all 4 finish at the end of k3 and
  evacuate mid-kernel.
- Phase B = pixels 512..1023 as 4 slices of 128 px, m-major per slice:
  groups retire staggered so outputs drain while the PE still computes.
  The last slice evacuates per-m (retirements are 213ns apart) for a
  short kernel tail.
- All input DMAs issue from SP in consumption order (HWDGE serializes
  descriptor generation at ~500ns per DMA, so issue order == bus order).
  W and x ship interleaved as combined [W-slab | x-chunk] row blocks of
  one DRAM tensor; the first k-chunk is split W-then-x so the PE starts
  ~300ns earlier.
- PSUM evacuation (fp32->fp16 cast, no bias) alternates between the
  Activation and Vector engines, one instruction per 512-px group
  (per-m for the last slice, on separate PSUM tiles -- Tile's WAR
  tracking is tile-granular and would otherwise serialize them against
  the next group's matmuls).
- Writeback: outa + slice s0 go out as plain HWDGE DMAs mid-kernel.
  Slices s1..s3 use pre-prepared SWDGE scatter-adds on four queues,
  fired by trigger_dma right after each slice's final evacuation: the
  post-evacuation tail is just transfer + completion-sem instead of
  HWDGE-issue + DGE-delay + transfer + sem (~1.2us saved).  The s3
  writeback splits m0-m2 / m3 so the final transfer is only 32KB.
  scatter-add ADDs into DRAM, so the target regions are zeroed by
  early DMAs; the idx table (identity, wrapped in 16 partitions) must
  be replicated into all eight 16-partition groups because each GPSIMD
  Q7 core reads its own group's copy on real hardware.
"""

import numpy as np

import concourse.mybir as mybir
import concourse.tile as tile
from concourse import bacc
from concourse.bass_utils import run_bass_kernel_spmd

P = 128          # SBUF partitions
C = 512          # model channels
HW = 1024        # pixels per image (32*32)
B = 8            # batch == number of cores
KO = C // P      # contraction chunks (4)
MO = C // P      # output-channel chunks (4)
PXA = 512        # phase A pixels (one PSUM bank wide)
NSB = 4          # phase B slices
PXS = (HW - PXA) // NSB   # pixels per phase B slice (128)

_FP32 = mybir.dt.float32
_FP16 = mybir.dt.float16

_ID = mybir.ActivationFunctionType.Identity


def _build_fp16(nc):
    # wxa[k*P + p, 0:C]    = W_eff.T[k*128+p, :]   (lhsT slab for k-chunk)
    # wxa[k*P + p, C + j]  = x_core[k*128+p, j]    j in [0, 512)
    wxa = nc.declare_dram_parameter("wxa", [KO * P, C + PXA], _FP16, isOutput=False)
    # xb[s*P + p, k*PXS+j] = x_core[k*128+p, 512+s*128+j]
    xb = nc.declare_dram_parameter("xb", [NSB * P, KO * PXS], _FP16, isOutput=False)
    # scatter-add idx table: identity map, wrapped in 16 partitions and
    # replicated into all 8 GPSIMD-core partition groups
    sidx = nc.declare_dram_parameter("sidx", [P, P // 16], mybir.dt.int16,
                                     isOutput=False)
    # outa[m*P + p, j]       = y[m*128+p, j]              j in [0, 512)
    outa = nc.declare_dram_parameter("outa", [MO * P, PXA], _FP16, isOutput=True)
    # outb[s*P + p, m*PXS+j] = y[m*128+p, 512+s*128+j]
    outb = nc.declare_dram_parameter("outb", [NSB * P, MO * PXS], _FP16, isOutput=True)

    with tile.TileContext(nc) as tc:
        with (
            tc.tile_pool(name="wpool", bufs=1) as wpool,
            tc.tile_pool(name="xpool", bufs=1) as xpool,
            tc.tile_pool(name="opool", bufs=1) as opool,
            tc.tile_pool(name="psum", bufs=8, space="PSUM") as psum_pool,
        ):
            wxa_sb = [wpool.tile([P, C + PXA], _FP16, tag=f"wxa{k}",
                                 name=f"wxa{k}") for k in range(KO)]
            xb_sb = [xpool.tile([P, 2, KO * PXS], _FP16, tag=f"xb{g}", name=f"xb{g}")
                     for g in range(2)]
            oa_sb = [opool.tile([P, 2, PXA], _FP16, tag=f"oa{h}", name=f"oa{h}")
                     for h in range(2)]
            # the last two slices stage as [P, 1, 512] to satisfy the
            # dma_scatter_add shape contract (partitions * idx-groups == 128)
            ob_sb = [opool.tile([P, 1, MO * PXS] if s >= NSB - 3 else [P, MO * PXS],
                                _FP16, tag=f"ob{s}", name=f"ob{s}")
                     for s in range(NSB)]
            z_sb = opool.tile([P, 2, MO * PXS], _FP16, tag="z")
            idx_sb = opool.tile([P, P // 16], mybir.dt.int16, tag="idx")

            # --- input DMA stream (SP queue; issue order == bus order) ----
            def wxa_dma(k):
                nc.sync.dma_start(wxa_sb[k][:], wxa[k * P:(k + 1) * P])

            def xb_dma(g):
                srcv = xb[2 * g * P:(2 * g + 2) * P]
                nc.sync.dma_start(
                    xb_sb[g][:], srcv.rearrange("(s p) c -> p s c", s=2))

            # k0 ships as two 128KB halves (W slab, then x chunk): smaller
            # transfers put the first matmul ~300ns earlier than one 256KB DMA
            nc.sync.dma_start(wxa_sb[0][:, :C], wxa[0:P, :C])
            nc.sync.dma_start(wxa_sb[0][:, C:], wxa[0:P, C:])
            wxa_dma(1)
            wxa_dma(2)
            wxa_dma(3)
            nc.sync.dma_start(idx_sb[:], sidx[:])
            xb_dma(0)
            xb_dma(1)

            # Slices s1..s3 write back via pre-prepared SWDGE scatter-adds
            # fired by trigger_dma: after each slice's final evacuation the
            # tail is just transfer + sem instead of HWDGE-issue + DGE +
            # transfer + sem.  The last slice splits into two half-width
            # scatters so the final transfer is 64KB.  scatter-add ADDs into
            # DRAM, so zero the regions first (off the critical path).
            nc.vector.memset(z_sb[:], 0.0)
            nc.sync.dma_start(outb[(NSB - 3) * P:(NSB - 2) * P], z_sb[:, 0, :])
            nc.sync.dma_start(
                outb[(NSB - 2) * P:NSB * P].rearrange("(s p) c -> p s c", s=2),
                z_sb[:])
            sc_sem = [nc.alloc_semaphore(f"sc_dma{q}") for q in range(4)]
            CUT = 3 * PXS          # s3 splits m0-m2 | m3 (the tail quarter)
            nc.gpsimd.dma_scatter_add(
                outb[(NSB - 3) * P:(NSB - 2) * P], ob_sb[NSB - 3][:],
                idx_sb[:], P, P, MO * PXS,
                prepare_only=True, sem=sc_sem[0], queue_num=0)
            nc.gpsimd.dma_scatter_add(
                outb[(NSB - 2) * P:(NSB - 1) * P], ob_sb[NSB - 2][:],
                idx_sb[:], P, P, MO * PXS,
                prepare_only=True, sem=sc_sem[1], queue_num=1)
            nc.gpsimd.dma_scatter_add(
                outb[(NSB - 1) * P:NSB * P, :CUT],
                ob_sb[NSB - 1][:, :, :CUT],
                idx_sb[:], P, P, CUT,
                elem_step=MO * PXS,
                prepare_only=True, sem=sc_sem[2], queue_num=2)
            nc.gpsimd.dma_scatter_add(
                outb[(NSB - 1) * P:NSB * P, CUT:],
                ob_sb[NSB - 1][:, :, CUT:],
                idx_sb[:], P, P, MO * PXS - CUT,
                elem_step=MO * PXS,
                prepare_only=True, sem=sc_sem[3], queue_num=3)

            def evac(dst, src, use_act):
                if use_act:
                    nc.scalar.activation(dst, src, _ID)
                else:
                    nc.vector.tensor_scalar_add(dst, src, 0.0)

            # --- phase A: k-outer, 4 wide PSUM groups -------------------
            ps_a = [psum_pool.tile([P, PXA], _FP32, tag="ps", name=f"psa{m}")
                    for m in range(MO)]
            for k in range(KO):
                for m in range(MO):
                    nc.tensor.matmul(
                        ps_a[m][:], lhsT=wxa_sb[k][:, m * P:(m + 1) * P],
                        rhs=wxa_sb[k][:, C:], start=(k == 0), stop=(k == KO - 1))

            for m in range(MO):
                evac(oa_sb[m // 2][:, m % 2, :], ps_a[m][:], use_act=(m % 2 == 0))
            for h in range(2):
                nc.sync.dma_start(
                    outa[2 * h * P:(2 * h + 2) * P].rearrange(
                        "(m p) j -> p m j", m=2),
                    oa_sb[h][:])

            # --- phase B: 4 slices of 128 px, m-major -------------------
            for s in range(NSB):
                g, sl = divmod(s, 2)
                if s != NSB - 1:
                    ps = psum_pool.tile([P, MO, PXS], _FP32, tag="ps",
                                       name=f"psb{s}")
                    for m in range(MO):
                        for k in range(KO):
                            nc.tensor.matmul(
                                ps[:, m, :], lhsT=wxa_sb[k][:, m * P:(m + 1) * P],
                                rhs=xb_sb[g][:, sl, k * PXS:(k + 1) * PXS],
                                start=(k == 0), stop=(k == KO - 1))
                    dst = ob_sb[s][:, 0, :] if s >= NSB - 3 else ob_sb[s][:]
                    evac(dst, ps[:], use_act=(s % 2 == 0))
                else:
                    # last slice: separate PSUM tiles per m-group so the
                    # per-m evacuations don't serialize against the next
                    # group's matmuls (tile-granular WAR tracking)
                    for m in range(MO):
                        psm = psum_pool.tile([P, PXS], _FP32, tag="ps",
                                            name=f"psb{s}m{m}")
                        for k in range(KO):
                            nc.tensor.matmul(
                                psm[:], lhsT=wxa_sb[k][:, m * P:(m + 1) * P],
                                rhs=xb_sb[g][:, sl, k * PXS:(k + 1) * PXS],
                                start=(k == 0), stop=(k == KO - 1))
                        evac(ob_sb[s][:, 0, m * PXS:(m + 1) * PXS], psm[:],
                             use_act=(m == 1))
                        if m == 2:
                            nc.gpsimd.trigger_dma(count=None, queue_num=2)
                    nc.gpsimd.trigger_dma(count=None, queue_num=3)
                if s < NSB - 3:
                    nc.sync.dma_start(outb[s * P:(s + 1) * P], ob_sb[s][:])
                elif s < NSB - 1:
                    nc.gpsimd.trigger_dma(count=None, queue_num=s - (NSB - 3))


def _build_bass(mode="fp16"):
    # Bacc (not plain Bass): its finalize() runs the legalization passes that
    # split multi-semaphore waits (TRN2 allows one sync wait per instruction).
    nc = bacc.Bacc(num_swdge_queues=2)
    _build_fp16(nc)
    nc.finalize()
    return nc


_NC_CACHE = {}


def _get_nc(mode):
    if mode not in _NC_CACHE:
        _NC_CACHE[mode] = _build_bass(mode)
    return _NC_CACHE[mode]


MODE = "fp16"


def _pack_w(w2d):
    # [C, C] (transposed W_eff: w2d[c, o]) -> [P, KO*C] with [p, ko, o] layout
    return np.ascontiguousarray(
        w2d.reshape(KO, P, C).transpose(1, 0, 2)).reshape(P, KO * C)


def kernel(x, qkv_w, qkv_b, out_w, out_b):
    x = np.asarray(x, dtype=np.float32)
    qkv_w = np.asarray(qkv_w, dtype=np.float32)
    qkv_b = np.asarray(qkv_b, dtype=np.float32)
    out_w = np.asarray(out_w, dtype=np.float32)
    out_b = np.asarray(out_b, dtype=np.float32)

    Bx, Cx, Hx, Wx = x.shape
    assert (Bx, Cx, Hx * Wx) == (B, C, HW), (x.shape,)

    # Host-side algebraic fusion (see module docstring).
    v_w = qkv_w[2 * C:3 * C]
    v_b = qkv_b[2 * C:3 * C]
    w_eff = out_w @ v_w                    # [C, C]
    b_eff = out_w @ v_b + out_b            # [C]

    xm = x.reshape(B, C, HW).astype(np.float16)
    wt = np.ascontiguousarray(w_eff.T).astype(np.float16)   # wt[c, o]

    # wxa[b, k*P+p, :] = [wt[k*128+p, :] | xm[b, k*128+p, 0:512]]
    wxa_host = np.empty((B, KO * P, C + PXA), np.float16)
    wxa_host[:, :, :C] = wt[None, :, :]
    wxa_host[:, :, C:] = xm[:, :, :PXA]
    # xb[b, s*P+p, k*PXS+j] = xm[b, k*128+p, PXA + s*PXS + j]
    xb_host = np.ascontiguousarray(
        xm[:, :, PXA:].reshape(B, KO, P, NSB, PXS)
        .transpose(0, 3, 2, 1, 4)).reshape(B, NSB * P, KO * PXS)

    sidx_host = np.zeros((P, P // 16), np.int16)
    for i in range(P):
        for c in range(P // 16):
            sidx_host[i % 16 + 16 * c, i // 16] = i

    nc = _get_nc(MODE)
    in_maps = [
        {"wxa": wxa_host[i], "xb": xb_host[i], "sidx": sidx_host}
        for i in range(B)
    ]
    res = run_bass_kernel_spmd(nc, in_maps, core_ids=list(range(B)))

    out_full = np.empty((B, C, HW), dtype=np.float32)
    for i in range(B):
        oa = np.asarray(res.results[i]["outa"], dtype=np.float32)
        ob = np.asarray(res.results[i]["outb"], dtype=np.float32)
        out_full[i, :, :PXA] = oa                       # [(m p), j] == [c, j]
        # ob[s*P+p, m*PXS+j] -> y[m*128+p, PXA+s*128+j]
        ob = ob.reshape(NSB, P, MO, PXS).transpose(2, 1, 0, 3).reshape(C, HW - PXA)
        out_full[i, :, PXA:] = ob
    out_full += b_eff[None, :, None]
    return np.ascontiguousarray(out_full.reshape(B, C, Hx, Wx))
